# revision 25
# baseline (speedup 1.0000x reference)
"""ProTCL-style dense MLP over a [B=16, NL=5000] cross-join, on 8 TRN2 cores.

Math (reference):
    P_e = seq @ Wp.T; L_e = lab @ Wl.T
    h   = relu(P_e @ W1p.T [+broadcast] L_e @ W1l.T + b1)   # [B, NL, O]
    h   = relu(h @ W2.T + b2)                               # [B, NL, O]
    out = (h @ W3.T + b3)[..., 0]                           # [B, NL]

Strategy:
  - Shard the label axis across 8 cores (625 labels each); host gathers.
  - Host folds (W1l @ Wl) into one matrix Wfl so the device does a single
    matmul for hl = lab @ Wfl.T, and folds the whole (tiny, B=16) sequence
    path into hpb = seq @ (W1p @ Wp).T + b1 on the host.
  - Device works feature-major for h: features on partitions, (b, label)
    sample columns on the free axis.
  - The dominant W2 matmul runs with h STATIONARY and W2 moving, so its
    output z lands sample-major [128 samples, 1024 features]. The W3
    dot-product then becomes a FREE-dim reduction, which the relu pass
    computes for free via the scalar engine's accum_out:
        W3 is folded into W2 on the host (W2'' = diag(W3) W2, rows permuted
        so w3>=0 features come first). Then
        w3_o*relu(y_o) = relu(z_o) for w3_o>=0, and -relu(-z_o) otherwise,
        so logits = sum(relu(z_pos)) - sum(relu(-z_neg)) — ACT passes with
        accum_out, no PE matvec and no bf16 rounding of y.
  - bf16 operands with fp32 PSUM accumulation.
"""

import numpy as np

B = 16
NL = 5000
D = 1024
O = 1024
N_CORES = 8
NLC = NL // N_CORES        # 625 labels per core
KT = O // 128              # 8 contraction tiles
SC = 512                   # sample columns per h-prep super-chunk
COLS = B * NLC             # 10000 sample columns per core
NRT = (COLS + 127) // 128  # 79 sample r-tiles (last one 16 wide)

_CACHE = {}


def _patch_tile_drain():
    """This container's walrus codegen rejects >1 sync wait per instruction.
    Tile freely attaches one wait per producer domain. Two patches:
      1. _commit_instruction: before committing an instruction carrying N>1
         waits, emit N-1 single-wait NOPs on the same engine (engines execute
         their stream in order, so semantics are identical).
      2. The tail drain (emitted after the commit hook) gets the same
         treatment via _drain_and_barrier."""
    import concourse.mybir as mybir
    import concourse.tile as tile
    from concourse.vector_clock import ScopedClock

    if getattr(tile.TileContext, "_drain_patch_applied", False):
        return

    _orig_commit = tile.TileContext._commit_instruction

    def _commit_instruction(self, inst, lazy_reg_writes: bool = True):
        si = getattr(inst, "sync_info", None)
        if si is not None and len(si.on_wait) > 1:
            waits = list(si.on_wait)
            for w in waits[:-1]:
                nop = mybir.InstNoOp(
                    name=self.nc.get_next_instruction_name(),
                    engine=inst.engine,
                    sync_info=mybir.SyncInfo(on_wait=[w], on_update=[]),
                    bass_nofuse=True,
                )
                self._add_instruction(nop)
            inst.sync_info = mybir.SyncInfo(
                on_wait=[waits[-1]], on_update=list(si.on_update)
            )
        return _orig_commit(self, inst, lazy_reg_writes)

    tile.TileContext._commit_instruction = _commit_instruction

    def _drain_and_barrier(self, tick_clock, wait_clock):
        probe = self.nc.sync.nop(nofuse=True)
        wait_clock.add_sem_waits(
            probe.ins, ScopedClock({None: tick_clock.global_clock})
        )
        si = probe.ins.sync_info
        waits = list(si.on_wait) if si is not None else []
        if len(waits) > 1:
            probe.ins.sync_info = mybir.SyncInfo(
                on_wait=waits[:1], on_update=list(si.on_update)
            )
            for w in waits[1:]:
                extra = self.nc.sync.nop(nofuse=True)
                extra.ins.sync_info = mybir.SyncInfo(on_wait=[w], on_update=[])
        self.nc.sync.drain()
        self.nc.all_engine_barrier()
        assert self.sems is not None
        popped = self.nc._tile_sem_poison_stack.pop()
        assert popped is self._sem_poison
        self.nc.clear_and_free_semaphores(list(self.sems.allocated().values()))
        self.nc.all_engine_barrier()

    tile.TileContext._drain_and_barrier = _drain_and_barrier
    tile.TileContext._drain_patch_applied = True


def _build_nc(cut, has_b2):
    """cut: number of w3>=0 features after the host permutation (static
    split point between the relu(+) and min(,0) accumulation regions).
    has_b2: emit the z += b2 pass (skipped when b2 is identically zero)."""
    import concourse.bass as bass
    import concourse.mybir as mybir
    import concourse.tile as tile

    _patch_tile_drain()

    f32 = mybir.dt.float32
    bf16 = mybir.dt.bfloat16
    Relu = mybir.ActivationFunctionType.Relu
    add = mybir.AluOpType.add
    amax = mybir.AluOpType.max
    sub = mybir.AluOpType.subtract

    nc = bass.Bass("TRN2")

    labT_d = nc.declare_dram_parameter("labT", [128, KT, NLC], bf16, isOutput=False)
    wfl_d = nc.declare_dram_parameter("wflT", [128, KT, 8, 128], bf16, isOutput=False)
    w2_d = nc.declare_dram_parameter("w2T", [128, KT, O], bf16, isOutput=False)
    hpb_d = nc.declare_dram_parameter("hpbT", [128, KT, B], f32, isOutput=False)
    b2_d = nc.declare_dram_parameter("b2bc", [128, O], f32, isOutput=False)
    out_d = nc.declare_dram_parameter("logits", [128, NRT], f32, isOutput=True)

    # per-quarter (256-wide) reduction regions: (offset, width, sign).
    # 256-wide matmuls avoid the full-bank (N=512) PSUM write penalty
    # (~0.51 ns/col vs ~0.41 measured).
    QW = 256
    NQ = O // QW
    q_regions = []
    for q in range(NQ):
        lo, hi = q * QW, (q + 1) * QW
        regs = []
        p_hi = min(hi, max(lo, cut))
        if p_hi > lo:
            regs.append((0, p_hi - lo, +1))
        n_lo = max(lo, min(hi, cut))
        if hi > n_lo:
            regs.append((n_lo - lo, hi - n_lo, -1))
        q_regions.append(regs)

    with tile.TileContext(nc) as tc:
        with (
            tc.tile_pool(name="const", bufs=1) as cpool,
            tc.tile_pool(name="h", bufs=3) as hpool,
            tc.tile_pool(name="scr", bufs=4) as spool,
            tc.tile_pool(name="acc", bufs=24) as apool,
            tc.tile_pool(name="ps", bufs=7, space="PSUM") as pspool,
        ):
            # Per-k tiles so dependencies (and DMAs) are fine-grained: the
            # first phase-1 matmul only waits for its own k-slice DMAs.
            labT = [cpool.tile([128, NLC], bf16, tag=f"labT{k}", name=f"labT{k}") for k in range(KT)]
            wfl = [cpool.tile([128, 8, 128], bf16, tag=f"wfl{k}", name=f"wfl{k}") for k in range(KT)]
            for k in range(KT):
                nc.sync.dma_start(labT[k][:], labT_d[:, k])
                nc.sync.dma_start(wfl[k][:], wfl_d[:, k])
            w2 = [cpool.tile([128, O], bf16, tag=f"w2{k}", name=f"w2{k}") for k in range(KT)]
            for k in range(KT):
                nc.sync.dma_start(w2[k][:], w2_d[:, k])
            hpb = cpool.tile([128, KT, B], f32)
            nc.sync.dma_start(hpb[:], hpb_d[:])
            if has_b2:
                b2bc = cpool.tile([128, O], f32)
                nc.sync.dma_start(b2bc[:], b2_d[:])

            logits_part = cpool.tile([128, NRT], f32)
            hlT = [cpool.tile([128, NLC], bf16, tag=f"hlT{k}", name=f"hlT{k}") for k in range(KT)]

            # ---- phase 0: PE warmup on junk data while input DMAs stream ----
            warm = cpool.tile([128, 512], bf16)
            nc.scalar.memzero(warm[:])
            wps = pspool.tile([128, 512], f32, tag="ps", name="warm_ps")
            for _ in range(34):
                nc.tensor.matmul(
                    wps[:, :500], lhsT=warm[:, :128], rhs=warm[:, :500], start=True, stop=True
                )

            # ---- phase 1: hlT[m][l] = (Wfl @ labT)[m-tile] ----
            # k-outer over half the m-tiles at a time (4 PSUM banks), so the
            # first matmul starts as soon as the k=0 slices have landed.
            for n0, w in ((0, 500), (500, NLC - 500)):
                for mhalf in (0, 4):
                    pss = [
                        pspool.tile([128, 512], f32, tag="ps", name=f"ps1_{n0}_{mhalf}_{i}")
                        for i in range(4)
                    ]
                    for k in range(KT):
                        for mi in range(4):
                            m = mhalf + mi
                            nc.tensor.matmul(
                                pss[mi][:, :w],
                                lhsT=wfl[k][:, m, :],
                                rhs=labT[k][:, n0 : n0 + w],
                                start=(k == 0),
                                stop=(k == KT - 1),
                            )
                    for mi in range(4):
                        m = mhalf + mi
                        if m % 2 == 0:
                            nc.vector.tensor_copy(hlT[m][:, n0 : n0 + w], pss[mi][:, :w])
                        else:
                            nc.scalar.copy(hlT[m][:, n0 : n0 + w], pss[mi][:, :w])

            # ---- phase 2: h stationary, W2'' moving; relu+reduce on ACT ----
            for sc0 in range(0, COLS, SC):
                w = min(SC, COLS - sc0)
                segs = []
                for b in range(sc0 // NLC, (sc0 + w - 1) // NLC + 1):
                    lo = max(sc0, b * NLC)
                    hi = min(sc0 + w, (b + 1) * NLC)
                    segs.append((b, lo, hi))

                h_sb = hpool.tile([128, KT, SC], bf16, tag="h", name=f"h_{sc0}")
                for k in range(KT):
                    for b, lo, hi in segs:
                        nc.vector.tensor_scalar(
                            h_sb[:, k, lo - sc0 : hi - sc0],
                            hlT[k][:, lo - b * NLC : hi - b * NLC],
                            hpb[:, k, b : b + 1],
                            0.0,
                            add,
                            amax,
                        )

                for r0 in range(0, w, 128):
                    m = min(128, w - r0)
                    rt = (sc0 + r0) // 128
                    accs = []
                    for q in range(NQ):
                        ps = pspool.tile(
                            [128, QW], f32, tag="ps", name=f"ps_{rt}_{q}"
                        )
                        for k in range(KT):
                            nc.tensor.matmul(
                                ps[:m, :],
                                lhsT=h_sb[:, k, r0 : r0 + m],
                                rhs=w2[k][:, q * QW : (q + 1) * QW],
                                start=(k == 0),
                                stop=(k == KT - 1),
                            )
                        if has_b2:
                            # z += b2'' (broadcast rows materialized host-side)
                            nc.vector.tensor_tensor(
                                ps[:m, :],
                                ps[:m, :],
                                b2bc[:m, q * QW : (q + 1) * QW],
                                add,
                            )
                        for off, wr, sign in q_regions[q]:
                            acc = apool.tile(
                                [128, 1], f32, tag="acc", name=f"acc_{rt}_{q}_{off}"
                            )
                            if sign > 0:
                                # sum(w3*relu(y)) over w3>=0 rows = sum(relu(z))
                                scr = spool.tile(
                                    [128, QW], bf16, tag="scr", name=f"scr_{rt}_{q}_{off}"
                                )
                                nc.scalar.activation(
                                    scr[:m, :wr],
                                    ps[:m, off : off + wr],
                                    Relu,
                                    accum_out=acc[:m],
                                )
                            else:
                                # sum over w3<0 rows = sum(min(z, 0))
                                scr = spool.tile(
                                    [128, QW], bf16, tag="vscr", name=f"vscr_{rt}_{q}_{off}"
                                )
                                nc.vector.tensor_scalar(
                                    scr[:m, :wr],
                                    ps[:m, off : off + wr],
                                    0.0,
                                    0.0,
                                    mybir.AluOpType.min,
                                    add,
                                    accum_out=acc[:m],
                                )
                            accs.append(acc)

                    # logits[rt] = sum of all accumulators
                    t = apool.tile([128, 1], f32, tag="acc", name=f"sum_{rt}")
                    nc.vector.tensor_tensor(t[:m], accs[0][:m], accs[1][:m], add)
                    for a in accs[2:-1]:
                        nc.vector.tensor_tensor(t[:m], t[:m], a[:m], add)
                    nc.vector.tensor_tensor(
                        logits_part[:m, rt : rt + 1], t[:m], accs[-1][:m], add
                    )
                    if rt == 47:
                        nc.sync.dma_start(out_d[:, :48], logits_part[:, :48])
                    elif rt == NRT - 1:
                        nc.sync.dma_start(out_d[:, 48:], logits_part[:, 48:])


    return nc


def prepare(
    sequence_embeddings,
    label_embeddings,
    Wp,
    Wl,
    W1,
    b1,
    W2,
    b2,
    W3,
    b3,
):
    """Host-side prep: returns (nc, in_maps, b3) ready for run_bass_kernel_spmd."""
    import ml_dtypes

    seq = np.asarray(sequence_embeddings, np.float32)
    lab = np.asarray(label_embeddings, np.float32)
    Wp = np.asarray(Wp, np.float32)
    Wl = np.asarray(Wl, np.float32)
    W1 = np.asarray(W1, np.float32)
    b1 = np.asarray(b1, np.float32)
    W2 = np.asarray(W2, np.float32)
    b2 = np.asarray(b2, np.float32)
    W3 = np.asarray(W3, np.float32)
    b3 = np.asarray(b3, np.float32)

    bf = ml_dtypes.bfloat16

    # Host-side algebraic folds (cheap: 2.1 GFLOP + 36 MFLOP).
    Wfl = W1[:, D:] @ Wl                       # [O, L_DIM]
    hpb = seq @ (W1[:, :D] @ Wp).T + b1        # [B, O]

    # Fold W3 into W2 and permute features so w3>=0 come first.
    w3v = W3[0]
    perm = np.concatenate([np.where(w3v >= 0)[0], np.where(w3v < 0)[0]])
    cut = int((w3v >= 0).sum())
    W2p = (W2 * w3v[:, None])[perm]            # [O, O]
    b2p = (b2 * w3v)[perm]                     # [O]

    wflT = np.ascontiguousarray(
        Wfl.T.reshape(KT, 128, 8, 128).transpose(1, 0, 2, 3)
    ).astype(bf)
    w2T = np.ascontiguousarray(W2p.T.reshape(KT, 128, O).transpose(1, 0, 2)).astype(bf)
    hpbT = np.ascontiguousarray(hpb.T.reshape(KT, 128, B).transpose(1, 0, 2)).astype(
        np.float32
    )
    b2bc = np.ascontiguousarray(np.broadcast_to(b2p, (128, O))).astype(np.float32)

    in_maps = []
    for c in range(N_CORES):
        shard = lab[c * NLC : (c + 1) * NLC]   # [NLC, L_DIM]
        labT = np.ascontiguousarray(
            shard.T.reshape(KT, 128, NLC).transpose(1, 0, 2)
        ).astype(bf)
        in_maps.append(
            {"labT": labT, "wflT": wflT, "w2T": w2T, "hpbT": hpbT, "b2bc": b2bc}
        )

    has_b2 = bool(np.any(b2p))
    key = ("nc", cut, has_b2)
    if key not in _CACHE:
        _CACHE[key] = _build_nc(cut, has_b2)
    nc = _CACHE[key]
    return nc, in_maps, b3


def gather(results, b3):
    """results[c]["logits"]: [128, NRT] with flat sample index rt*128+p."""
    parts = [
        results[c]["logits"].T.ravel()[:COLS].reshape(B, NLC)
        for c in range(N_CORES)
    ]
    out = np.concatenate(parts, axis=1) + b3[0]
    return out.astype(np.float32)


def kernel(**inputs):
    from concourse.bass_utils import run_bass_kernel_spmd

    nc, in_maps, b3 = prepare(**inputs)
    res = run_bass_kernel_spmd(nc, in_maps, core_ids=list(range(N_CORES)))
    return gather(res.results, b3)


# revision 26
# speedup vs baseline: 1.0174x; 1.0174x over previous
"""ProTCL-style dense MLP over a [B=16, NL=5000] cross-join, on 8 TRN2 cores.

Math (reference):
    P_e = seq @ Wp.T; L_e = lab @ Wl.T
    h   = relu(P_e @ W1p.T [+broadcast] L_e @ W1l.T + b1)   # [B, NL, O]
    h   = relu(h @ W2.T + b2)                               # [B, NL, O]
    out = (h @ W3.T + b3)[..., 0]                           # [B, NL]

Strategy:
  - Shard the label axis across 8 cores (625 labels each); host gathers.
  - Host folds (W1l @ Wl) into one matrix Wfl so the device does a single
    matmul for hl = lab @ Wfl.T, and folds the whole (tiny, B=16) sequence
    path into hpb = seq @ (W1p @ Wp).T + b1 on the host.
  - Device works feature-major for h: features on partitions, (b, label)
    sample columns on the free axis.
  - The dominant W2 matmul runs with h STATIONARY and W2 moving, so its
    output z lands sample-major [128 samples, 1024 features]. The W3
    dot-product then becomes a FREE-dim reduction, which the relu pass
    computes for free via the scalar engine's accum_out:
        W3 is folded into W2 on the host (W2'' = diag(W3) W2, rows permuted
        so w3>=0 features come first). Then
        w3_o*relu(y_o) = relu(z_o) for w3_o>=0, and -relu(-z_o) otherwise,
        so logits = sum(relu(z_pos)) - sum(relu(-z_neg)) — ACT passes with
        accum_out, no PE matvec and no bf16 rounding of y.
  - bf16 operands with fp32 PSUM accumulation.
"""

import numpy as np

B = 16
NL = 5000
D = 1024
O = 1024
N_CORES = 8
NLC = NL // N_CORES        # 625 labels per core
KT = O // 128              # 8 contraction tiles
SC = 512                   # sample columns per h-prep super-chunk
COLS = B * NLC             # 10000 sample columns per core
COLS_DEV = (COLS // 128) * 128  # 9984 columns on device (full 128-wide r-tiles)
NRT = COLS_DEV // 128      # 78 sample r-tiles
# the ragged 16-column tail (b=15, last labels of each shard) is computed on
# the host in fp32 — 0.5 GFLOP total, avoids a 97%-idle PE tile per core

_CACHE = {}


def _patch_tile_drain():
    """This container's walrus codegen rejects >1 sync wait per instruction.
    Tile freely attaches one wait per producer domain. Two patches:
      1. _commit_instruction: before committing an instruction carrying N>1
         waits, emit N-1 single-wait NOPs on the same engine (engines execute
         their stream in order, so semantics are identical).
      2. The tail drain (emitted after the commit hook) gets the same
         treatment via _drain_and_barrier."""
    import concourse.mybir as mybir
    import concourse.tile as tile
    from concourse.vector_clock import ScopedClock

    if getattr(tile.TileContext, "_drain_patch_applied", False):
        return

    _orig_commit = tile.TileContext._commit_instruction

    def _commit_instruction(self, inst, lazy_reg_writes: bool = True):
        si = getattr(inst, "sync_info", None)
        if si is not None and len(si.on_wait) > 1:
            waits = list(si.on_wait)
            for w in waits[:-1]:
                nop = mybir.InstNoOp(
                    name=self.nc.get_next_instruction_name(),
                    engine=inst.engine,
                    sync_info=mybir.SyncInfo(on_wait=[w], on_update=[]),
                    bass_nofuse=True,
                )
                self._add_instruction(nop)
            inst.sync_info = mybir.SyncInfo(
                on_wait=[waits[-1]], on_update=list(si.on_update)
            )
        return _orig_commit(self, inst, lazy_reg_writes)

    tile.TileContext._commit_instruction = _commit_instruction

    def _drain_and_barrier(self, tick_clock, wait_clock):
        probe = self.nc.sync.nop(nofuse=True)
        wait_clock.add_sem_waits(
            probe.ins, ScopedClock({None: tick_clock.global_clock})
        )
        si = probe.ins.sync_info
        waits = list(si.on_wait) if si is not None else []
        if len(waits) > 1:
            probe.ins.sync_info = mybir.SyncInfo(
                on_wait=waits[:1], on_update=list(si.on_update)
            )
            for w in waits[1:]:
                extra = self.nc.sync.nop(nofuse=True)
                extra.ins.sync_info = mybir.SyncInfo(on_wait=[w], on_update=[])
        self.nc.sync.drain()
        self.nc.all_engine_barrier()
        assert self.sems is not None
        popped = self.nc._tile_sem_poison_stack.pop()
        assert popped is self._sem_poison
        self.nc.clear_and_free_semaphores(list(self.sems.allocated().values()))
        self.nc.all_engine_barrier()

    tile.TileContext._drain_and_barrier = _drain_and_barrier
    tile.TileContext._drain_patch_applied = True


def _build_nc(cut, has_b2):
    """cut: number of w3>=0 features after the host permutation (static
    split point between the relu(+) and min(,0) accumulation regions).
    has_b2: emit the z += b2 pass (skipped when b2 is identically zero)."""
    import concourse.bass as bass
    import concourse.mybir as mybir
    import concourse.tile as tile

    _patch_tile_drain()

    f32 = mybir.dt.float32
    bf16 = mybir.dt.bfloat16
    Relu = mybir.ActivationFunctionType.Relu
    add = mybir.AluOpType.add
    amax = mybir.AluOpType.max
    sub = mybir.AluOpType.subtract

    nc = bass.Bass("TRN2")

    labT_d = nc.declare_dram_parameter("labT", [128, KT, NLC], bf16, isOutput=False)
    wfl_d = nc.declare_dram_parameter("wflT", [128, KT, 8, 128], bf16, isOutput=False)
    w2_d = nc.declare_dram_parameter("w2T", [128, KT, O], bf16, isOutput=False)
    hpb_d = nc.declare_dram_parameter("hpbT", [128, KT, B], f32, isOutput=False)
    b2_d = nc.declare_dram_parameter("b2bc", [128, O], f32, isOutput=False)
    out_d = nc.declare_dram_parameter("logits", [128, NRT], f32, isOutput=True)

    # per-quarter (256-wide) reduction regions: (offset, width, sign).
    # 256-wide matmuls avoid the full-bank (N=512) PSUM write penalty
    # (~0.51 ns/col vs ~0.41 measured).
    QW = 256
    NQ = O // QW
    q_regions = []
    for q in range(NQ):
        lo, hi = q * QW, (q + 1) * QW
        regs = []
        p_hi = min(hi, max(lo, cut))
        if p_hi > lo:
            regs.append((0, p_hi - lo, +1))
        n_lo = max(lo, min(hi, cut))
        if hi > n_lo:
            regs.append((n_lo - lo, hi - n_lo, -1))
        q_regions.append(regs)

    with tile.TileContext(nc) as tc:
        with (
            tc.tile_pool(name="const", bufs=1) as cpool,
            tc.tile_pool(name="h", bufs=3) as hpool,
            tc.tile_pool(name="scr", bufs=4) as spool,
            tc.tile_pool(name="acc", bufs=24) as apool,
            tc.tile_pool(name="ps", bufs=7, space="PSUM") as pspool,
        ):
            # Per-k tiles so dependencies (and DMAs) are fine-grained: the
            # first phase-1 matmul only waits for its own k-slice DMAs.
            labT = [cpool.tile([128, NLC], bf16, tag=f"labT{k}", name=f"labT{k}") for k in range(KT)]
            wfl = [cpool.tile([128, 8, 128], bf16, tag=f"wfl{k}", name=f"wfl{k}") for k in range(KT)]
            for k in range(KT):
                nc.sync.dma_start(labT[k][:], labT_d[:, k])
                nc.sync.dma_start(wfl[k][:], wfl_d[:, k])
            w2 = [cpool.tile([128, O], bf16, tag=f"w2{k}", name=f"w2{k}") for k in range(KT)]
            for k in range(KT):
                nc.sync.dma_start(w2[k][:], w2_d[:, k])
            hpb = cpool.tile([128, KT, B], f32)
            nc.sync.dma_start(hpb[:], hpb_d[:])
            if has_b2:
                b2bc = cpool.tile([128, O], f32)
                nc.sync.dma_start(b2bc[:], b2_d[:])

            logits_part = cpool.tile([128, NRT], f32)
            hlT = [cpool.tile([128, NLC], bf16, tag=f"hlT{k}", name=f"hlT{k}") for k in range(KT)]

            # ---- phase 0: PE warmup on junk data while input DMAs stream ----
            warm = cpool.tile([128, 512], bf16)
            nc.scalar.memzero(warm[:])
            wps = pspool.tile([128, 512], f32, tag="ps", name="warm_ps")
            for _ in range(34):
                nc.tensor.matmul(
                    wps[:, :500], lhsT=warm[:, :128], rhs=warm[:, :500], start=True, stop=True
                )

            # ---- phase 1: hlT[m][l] = (Wfl @ labT)[m-tile] ----
            # k-outer over half the m-tiles at a time (4 PSUM banks), so the
            # first matmul starts as soon as the k=0 slices have landed.
            for n0, w in ((0, 500), (500, NLC - 500)):
                for mhalf in (0, 4):
                    pss = [
                        pspool.tile([128, 512], f32, tag="ps", name=f"ps1_{n0}_{mhalf}_{i}")
                        for i in range(4)
                    ]
                    for k in range(KT):
                        for mi in range(4):
                            m = mhalf + mi
                            nc.tensor.matmul(
                                pss[mi][:, :w],
                                lhsT=wfl[k][:, m, :],
                                rhs=labT[k][:, n0 : n0 + w],
                                start=(k == 0),
                                stop=(k == KT - 1),
                            )
                    for mi in range(4):
                        m = mhalf + mi
                        if m % 2 == 0:
                            nc.vector.tensor_copy(hlT[m][:, n0 : n0 + w], pss[mi][:, :w])
                        else:
                            nc.scalar.copy(hlT[m][:, n0 : n0 + w], pss[mi][:, :w])

            # ---- phase 2: h stationary, W2'' moving; relu+reduce on ACT ----
            for sc0 in range(0, COLS_DEV, SC):
                w = min(SC, COLS_DEV - sc0)
                segs = []
                for b in range(sc0 // NLC, (sc0 + w - 1) // NLC + 1):
                    lo = max(sc0, b * NLC)
                    hi = min(sc0 + w, (b + 1) * NLC)
                    segs.append((b, lo, hi))

                h_sb = hpool.tile([128, KT, SC], bf16, tag="h", name=f"h_{sc0}")
                for k in range(KT):
                    for b, lo, hi in segs:
                        nc.vector.tensor_scalar(
                            h_sb[:, k, lo - sc0 : hi - sc0],
                            hlT[k][:, lo - b * NLC : hi - b * NLC],
                            hpb[:, k, b : b + 1],
                            0.0,
                            add,
                            amax,
                        )

                for r0 in range(0, w, 128):
                    m = min(128, w - r0)
                    rt = (sc0 + r0) // 128
                    accs = []
                    for q in range(NQ):
                        ps = pspool.tile(
                            [128, QW], f32, tag="ps", name=f"ps_{rt}_{q}"
                        )
                        for k in range(KT):
                            nc.tensor.matmul(
                                ps[:m, :],
                                lhsT=h_sb[:, k, r0 : r0 + m],
                                rhs=w2[k][:, q * QW : (q + 1) * QW],
                                start=(k == 0),
                                stop=(k == KT - 1),
                            )
                        if has_b2:
                            # z += b2'' (broadcast rows materialized host-side)
                            nc.vector.tensor_tensor(
                                ps[:m, :],
                                ps[:m, :],
                                b2bc[:m, q * QW : (q + 1) * QW],
                                add,
                            )
                        for off, wr, sign in q_regions[q]:
                            acc = apool.tile(
                                [128, 1], f32, tag="acc", name=f"acc_{rt}_{q}_{off}"
                            )
                            if sign > 0:
                                # sum(w3*relu(y)) over w3>=0 rows = sum(relu(z))
                                scr = spool.tile(
                                    [128, QW], bf16, tag="scr", name=f"scr_{rt}_{q}_{off}"
                                )
                                nc.scalar.activation(
                                    scr[:m, :wr],
                                    ps[:m, off : off + wr],
                                    Relu,
                                    accum_out=acc[:m],
                                )
                            else:
                                # sum over w3<0 rows = sum(min(z, 0))
                                scr = spool.tile(
                                    [128, QW], bf16, tag="vscr", name=f"vscr_{rt}_{q}_{off}"
                                )
                                nc.vector.tensor_scalar(
                                    scr[:m, :wr],
                                    ps[:m, off : off + wr],
                                    0.0,
                                    0.0,
                                    mybir.AluOpType.min,
                                    add,
                                    accum_out=acc[:m],
                                )
                            accs.append(acc)

                    # logits[rt] = sum of all accumulators
                    t = apool.tile([128, 1], f32, tag="acc", name=f"sum_{rt}")
                    nc.vector.tensor_tensor(t[:m], accs[0][:m], accs[1][:m], add)
                    for a in accs[2:-1]:
                        nc.vector.tensor_tensor(t[:m], t[:m], a[:m], add)
                    nc.vector.tensor_tensor(
                        logits_part[:m, rt : rt + 1], t[:m], accs[-1][:m], add
                    )
                    if rt == 47:
                        nc.sync.dma_start(out_d[:, :48], logits_part[:, :48])
                    elif rt == NRT - 1:
                        nc.sync.dma_start(out_d[:, 48:], logits_part[:, 48:])


    return nc


def prepare(
    sequence_embeddings,
    label_embeddings,
    Wp,
    Wl,
    W1,
    b1,
    W2,
    b2,
    W3,
    b3,
):
    """Host-side prep: returns (nc, in_maps, b3) ready for run_bass_kernel_spmd."""
    import ml_dtypes

    seq = np.asarray(sequence_embeddings, np.float32)
    lab = np.asarray(label_embeddings, np.float32)
    Wp = np.asarray(Wp, np.float32)
    Wl = np.asarray(Wl, np.float32)
    W1 = np.asarray(W1, np.float32)
    b1 = np.asarray(b1, np.float32)
    W2 = np.asarray(W2, np.float32)
    b2 = np.asarray(b2, np.float32)
    W3 = np.asarray(W3, np.float32)
    b3 = np.asarray(b3, np.float32)

    bf = ml_dtypes.bfloat16

    # Host-side algebraic folds (cheap: 2.1 GFLOP + 36 MFLOP).
    Wfl = W1[:, D:] @ Wl                       # [O, L_DIM]
    hpb = seq @ (W1[:, :D] @ Wp).T + b1        # [B, O]

    # Fold W3 into W2 and permute features so w3>=0 come first.
    w3v = W3[0]
    perm = np.concatenate([np.where(w3v >= 0)[0], np.where(w3v < 0)[0]])
    cut = int((w3v >= 0).sum())
    W2p = (W2 * w3v[:, None])[perm]            # [O, O]
    b2p = (b2 * w3v)[perm]                     # [O]

    wflT = np.ascontiguousarray(
        Wfl.T.reshape(KT, 128, 8, 128).transpose(1, 0, 2, 3)
    ).astype(bf)
    w2T = np.ascontiguousarray(W2p.T.reshape(KT, 128, O).transpose(1, 0, 2)).astype(bf)
    hpbT = np.ascontiguousarray(hpb.T.reshape(KT, 128, B).transpose(1, 0, 2)).astype(
        np.float32
    )
    b2bc = np.ascontiguousarray(np.broadcast_to(b2p, (128, O))).astype(np.float32)

    in_maps = []
    for c in range(N_CORES):
        shard = lab[c * NLC : (c + 1) * NLC]   # [NLC, L_DIM]
        labT = np.ascontiguousarray(
            shard.T.reshape(KT, 128, NLC).transpose(1, 0, 2)
        ).astype(bf)
        in_maps.append(
            {"labT": labT, "wflT": wflT, "w2T": w2T, "hpbT": hpbT, "b2bc": b2bc}
        )

    has_b2 = bool(np.any(b2p))
    key = ("nc", cut, has_b2)
    if key not in _CACHE:
        _CACHE[key] = _build_nc(cut, has_b2)
    nc = _CACHE[key]

    # Host-computed ragged tail: flat cols [COLS_DEV, COLS) of each core,
    # i.e. (b=B-1, last labels of the shard). fp32 exact.
    Wfl32, W2_32, W3_32 = Wfl, W2, W3
    tails = []
    nt = COLS - COLS_DEV
    b_last = COLS_DEV // NLC          # batch row of the tail region
    l0 = COLS_DEV - b_last * NLC      # first label offset in shard
    for c in range(N_CORES):
        shard = lab[c * NLC : (c + 1) * NLC]
        hl_t = shard[l0 : l0 + nt] @ Wfl32.T          # [nt, O]
        h_t = np.maximum(hl_t + hpb[b_last], 0.0)
        y_t = np.maximum(h_t @ W2_32.T + b2, 0.0)
        tails.append((y_t @ W3_32.T)[:, 0].astype(np.float32))
    return nc, in_maps, b3, tails


def gather(results, b3, tails):
    """results[c]["logits"]: [128, NRT], flat sample index rt*128+p; tails[c]
    covers the last COLS-COLS_DEV flat columns (host-computed)."""
    parts = []
    for c in range(N_CORES):
        flat = np.empty(COLS, np.float32)
        flat[:COLS_DEV] = results[c]["logits"].T.ravel()
        flat[COLS_DEV:] = tails[c]
        parts.append(flat.reshape(B, NLC))
    out = np.concatenate(parts, axis=1) + b3[0]
    return out.astype(np.float32)


def kernel(**inputs):
    from concourse.bass_utils import run_bass_kernel_spmd

    nc, in_maps, b3, tails = prepare(**inputs)
    res = run_bass_kernel_spmd(nc, in_maps, core_ids=list(range(N_CORES)))
    return gather(res.results, b3, tails)


# revision 27
# speedup vs baseline: 1.0226x; 1.0051x over previous
"""ProTCL-style dense MLP over a [B=16, NL=5000] cross-join, on 8 TRN2 cores.

Math (reference):
    P_e = seq @ Wp.T; L_e = lab @ Wl.T
    h   = relu(P_e @ W1p.T [+broadcast] L_e @ W1l.T + b1)   # [B, NL, O]
    h   = relu(h @ W2.T + b2)                               # [B, NL, O]
    out = (h @ W3.T + b3)[..., 0]                           # [B, NL]

Strategy:
  - Shard the label axis across 8 cores (625 labels each); host gathers.
  - Host folds (W1l @ Wl) into one matrix Wfl so the device does a single
    matmul for hl = lab @ Wfl.T, and folds the whole (tiny, B=16) sequence
    path into hpb = seq @ (W1p @ Wp).T + b1 on the host.
  - Device works feature-major for h: features on partitions, (b, label)
    sample columns on the free axis.
  - The dominant W2 matmul runs with h STATIONARY and W2 moving, so its
    output z lands sample-major [128 samples, 1024 features]. The W3
    dot-product then becomes a FREE-dim reduction, which the relu pass
    computes for free via the scalar engine's accum_out:
        W3 is folded into W2 on the host (W2'' = diag(W3) W2, rows permuted
        so w3>=0 features come first). Then
        w3_o*relu(y_o) = relu(z_o) for w3_o>=0, and -relu(-z_o) otherwise,
        so logits = sum(relu(z_pos)) - sum(relu(-z_neg)) — ACT passes with
        accum_out, no PE matvec and no bf16 rounding of y.
  - bf16 operands with fp32 PSUM accumulation.
"""

import numpy as np

B = 16
NL = 5000
D = 1024
O = 1024
N_CORES = 8
NLC = NL // N_CORES        # 625 labels per core
KT = O // 128              # 8 contraction tiles
SC = 512                   # sample columns per h-prep super-chunk
COLS = B * NLC             # 10000 sample columns per core
COLS_DEV = (COLS // 128) * 128  # 9984 columns on device (full 128-wide r-tiles)
NRT = COLS_DEV // 128      # 78 sample r-tiles
# the ragged 16-column tail (b=15, last labels of each shard) is computed on
# the host in fp32 — 0.5 GFLOP total, avoids a 97%-idle PE tile per core

_CACHE = {}


def _patch_tile_drain():
    """This container's walrus codegen rejects >1 sync wait per instruction.
    Tile freely attaches one wait per producer domain. Two patches:
      1. _commit_instruction: before committing an instruction carrying N>1
         waits, emit N-1 single-wait NOPs on the same engine (engines execute
         their stream in order, so semantics are identical).
      2. The tail drain (emitted after the commit hook) gets the same
         treatment via _drain_and_barrier."""
    import concourse.mybir as mybir
    import concourse.tile as tile
    from concourse.vector_clock import ScopedClock

    if getattr(tile.TileContext, "_drain_patch_applied", False):
        return

    _orig_commit = tile.TileContext._commit_instruction

    def _commit_instruction(self, inst, lazy_reg_writes: bool = True):
        si = getattr(inst, "sync_info", None)
        if si is not None and len(si.on_wait) > 1:
            waits = list(si.on_wait)
            for w in waits[:-1]:
                nop = mybir.InstNoOp(
                    name=self.nc.get_next_instruction_name(),
                    engine=inst.engine,
                    sync_info=mybir.SyncInfo(on_wait=[w], on_update=[]),
                    bass_nofuse=True,
                )
                self._add_instruction(nop)
            inst.sync_info = mybir.SyncInfo(
                on_wait=[waits[-1]], on_update=list(si.on_update)
            )
        return _orig_commit(self, inst, lazy_reg_writes)

    tile.TileContext._commit_instruction = _commit_instruction

    def _drain_and_barrier(self, tick_clock, wait_clock):
        probe = self.nc.sync.nop(nofuse=True)
        wait_clock.add_sem_waits(
            probe.ins, ScopedClock({None: tick_clock.global_clock})
        )
        si = probe.ins.sync_info
        waits = list(si.on_wait) if si is not None else []
        if len(waits) > 1:
            probe.ins.sync_info = mybir.SyncInfo(
                on_wait=waits[:1], on_update=list(si.on_update)
            )
            for w in waits[1:]:
                extra = self.nc.sync.nop(nofuse=True)
                extra.ins.sync_info = mybir.SyncInfo(on_wait=[w], on_update=[])
        self.nc.sync.drain()
        self.nc.all_engine_barrier()
        assert self.sems is not None
        popped = self.nc._tile_sem_poison_stack.pop()
        assert popped is self._sem_poison
        self.nc.clear_and_free_semaphores(list(self.sems.allocated().values()))
        self.nc.all_engine_barrier()

    tile.TileContext._drain_and_barrier = _drain_and_barrier
    tile.TileContext._drain_patch_applied = True


def _build_nc(cut, has_b2):
    """cut: number of w3>=0 features after the host permutation (static
    split point between the relu(+) and min(,0) accumulation regions).
    has_b2: emit the z += b2 pass (skipped when b2 is identically zero)."""
    import concourse.bass as bass
    import concourse.mybir as mybir
    import concourse.tile as tile

    _patch_tile_drain()

    f32 = mybir.dt.float32
    bf16 = mybir.dt.bfloat16
    Relu = mybir.ActivationFunctionType.Relu
    add = mybir.AluOpType.add
    amax = mybir.AluOpType.max
    sub = mybir.AluOpType.subtract

    nc = bass.Bass("TRN2")

    labT_d = nc.declare_dram_parameter("labT", [128, KT, NLC], bf16, isOutput=False)
    wfl_d = nc.declare_dram_parameter("wflT", [128, KT, 8, 128], bf16, isOutput=False)
    w2_d = nc.declare_dram_parameter("w2T", [128, KT, O], bf16, isOutput=False)
    hpb_d = nc.declare_dram_parameter("hpbT", [128, KT, B], f32, isOutput=False)
    b2_d = nc.declare_dram_parameter("b2bc", [128, O], f32, isOutput=False)
    out_d = nc.declare_dram_parameter("logits", [128, NRT], f32, isOutput=True)

    # per-quarter (256-wide) reduction regions: (offset, width, sign).
    # 256-wide matmuls avoid the full-bank (N=512) PSUM write penalty
    # (~0.51 ns/col vs ~0.41 measured).
    QW = 256
    NQ = O // QW
    q_regions = []
    for q in range(NQ):
        lo, hi = q * QW, (q + 1) * QW
        regs = []
        p_hi = min(hi, max(lo, cut))
        if p_hi > lo:
            regs.append((0, p_hi - lo, +1))
        n_lo = max(lo, min(hi, cut))
        if hi > n_lo:
            regs.append((n_lo - lo, hi - n_lo, -1))
        q_regions.append(regs)

    with tile.TileContext(nc) as tc:
        with (
            tc.tile_pool(name="const", bufs=1) as cpool,
            tc.tile_pool(name="h", bufs=3) as hpool,
            tc.tile_pool(name="scr", bufs=4) as spool,
            tc.tile_pool(name="acc", bufs=24) as apool,
            tc.tile_pool(name="ps", bufs=7, space="PSUM") as pspool,
        ):
            # Per-k tiles so dependencies (and DMAs) are fine-grained: the
            # first phase-1 matmul only waits for its own k-slice DMAs.
            labT = [cpool.tile([128, NLC], bf16, tag=f"labT{k}", name=f"labT{k}") for k in range(KT)]
            wfl = [cpool.tile([128, 8, 128], bf16, tag=f"wfl{k}", name=f"wfl{k}") for k in range(KT)]
            for k in range(KT):
                nc.sync.dma_start(labT[k][:], labT_d[:, k])
                nc.sync.dma_start(wfl[k][:], wfl_d[:, k])
            w2 = [cpool.tile([128, O], bf16, tag=f"w2{k}", name=f"w2{k}") for k in range(KT)]
            for k in range(KT):
                nc.sync.dma_start(w2[k][:], w2_d[:, k])
            hpb = cpool.tile([128, KT, B], f32)
            nc.sync.dma_start(hpb[:], hpb_d[:])
            if has_b2:
                b2bc = cpool.tile([128, O], f32)
                nc.sync.dma_start(b2bc[:], b2_d[:])

            logits_part = cpool.tile([128, NRT], f32)
            hlT = [cpool.tile([128, NLC], bf16, tag=f"hlT{k}", name=f"hlT{k}") for k in range(KT)]

            # ---- phase 0: PE warmup while the remaining DMAs stream ----
            # Uses labT[0] (the first DMA to land, ~2.5us) as junk operands so
            # nothing has to be zero-filled first; output PSUM is never read.
            # ~16 matmuls = ~3.4us keeps the HAM window busy so phase 1 starts
            # at full clock as soon as wfl[0] arrives.
            wps = pspool.tile([128, 512], f32, tag="ps", name="warm_ps")
            for _ in range(16):
                nc.tensor.matmul(
                    wps[:, :500],
                    lhsT=labT[0][:, :128],
                    rhs=labT[0][:, :500],
                    start=True,
                    stop=True,
                )

            # ---- phase 1: hlT[m][l] = (Wfl @ labT)[m-tile] ----
            # k-outer over half the m-tiles at a time (4 PSUM banks), so the
            # first matmul starts as soon as the k=0 slices have landed.
            for n0, w in ((0, 500), (500, NLC - 500)):
                for mhalf in (0, 4):
                    pss = [
                        pspool.tile([128, 512], f32, tag="ps", name=f"ps1_{n0}_{mhalf}_{i}")
                        for i in range(4)
                    ]
                    for k in range(KT):
                        for mi in range(4):
                            m = mhalf + mi
                            nc.tensor.matmul(
                                pss[mi][:, :w],
                                lhsT=wfl[k][:, m, :],
                                rhs=labT[k][:, n0 : n0 + w],
                                start=(k == 0),
                                stop=(k == KT - 1),
                            )
                    for mi in range(4):
                        m = mhalf + mi
                        if m % 2 == 0:
                            nc.vector.tensor_copy(hlT[m][:, n0 : n0 + w], pss[mi][:, :w])
                        else:
                            nc.scalar.copy(hlT[m][:, n0 : n0 + w], pss[mi][:, :w])

            # ---- phase 2: h stationary, W2'' moving; relu+reduce on ACT ----
            for sc0 in range(0, COLS_DEV, SC):
                w = min(SC, COLS_DEV - sc0)
                segs = []
                for b in range(sc0 // NLC, (sc0 + w - 1) // NLC + 1):
                    lo = max(sc0, b * NLC)
                    hi = min(sc0 + w, (b + 1) * NLC)
                    segs.append((b, lo, hi))

                h_sb = hpool.tile([128, KT, SC], bf16, tag="h", name=f"h_{sc0}")
                for k in range(KT):
                    for b, lo, hi in segs:
                        nc.vector.tensor_scalar(
                            h_sb[:, k, lo - sc0 : hi - sc0],
                            hlT[k][:, lo - b * NLC : hi - b * NLC],
                            hpb[:, k, b : b + 1],
                            0.0,
                            add,
                            amax,
                        )

                for r0 in range(0, w, 128):
                    m = min(128, w - r0)
                    rt = (sc0 + r0) // 128
                    accs = []
                    for q in range(NQ):
                        ps = pspool.tile(
                            [128, QW], f32, tag="ps", name=f"ps_{rt}_{q}"
                        )
                        for k in range(KT):
                            nc.tensor.matmul(
                                ps[:m, :],
                                lhsT=h_sb[:, k, r0 : r0 + m],
                                rhs=w2[k][:, q * QW : (q + 1) * QW],
                                start=(k == 0),
                                stop=(k == KT - 1),
                            )
                        if has_b2:
                            # z += b2'' (broadcast rows materialized host-side)
                            nc.vector.tensor_tensor(
                                ps[:m, :],
                                ps[:m, :],
                                b2bc[:m, q * QW : (q + 1) * QW],
                                add,
                            )
                        for off, wr, sign in q_regions[q]:
                            acc = apool.tile(
                                [128, 1], f32, tag="acc", name=f"acc_{rt}_{q}_{off}"
                            )
                            if sign > 0:
                                # sum(w3*relu(y)) over w3>=0 rows = sum(relu(z))
                                scr = spool.tile(
                                    [128, QW], bf16, tag="scr", name=f"scr_{rt}_{q}_{off}"
                                )
                                nc.scalar.activation(
                                    scr[:m, :wr],
                                    ps[:m, off : off + wr],
                                    Relu,
                                    accum_out=acc[:m],
                                )
                            else:
                                # sum over w3<0 rows = sum(min(z, 0))
                                scr = spool.tile(
                                    [128, QW], bf16, tag="vscr", name=f"vscr_{rt}_{q}_{off}"
                                )
                                nc.vector.tensor_scalar(
                                    scr[:m, :wr],
                                    ps[:m, off : off + wr],
                                    0.0,
                                    0.0,
                                    mybir.AluOpType.min,
                                    add,
                                    accum_out=acc[:m],
                                )
                            accs.append(acc)

                    # logits[rt] = sum of all accumulators
                    t = apool.tile([128, 1], f32, tag="acc", name=f"sum_{rt}")
                    nc.vector.tensor_tensor(t[:m], accs[0][:m], accs[1][:m], add)
                    for a in accs[2:-1]:
                        nc.vector.tensor_tensor(t[:m], t[:m], a[:m], add)
                    nc.vector.tensor_tensor(
                        logits_part[:m, rt : rt + 1], t[:m], accs[-1][:m], add
                    )
                    if rt == 47:
                        nc.sync.dma_start(out_d[:, :48], logits_part[:, :48])
                    elif rt == NRT - 1:
                        nc.sync.dma_start(out_d[:, 48:], logits_part[:, 48:])


    return nc


def prepare(
    sequence_embeddings,
    label_embeddings,
    Wp,
    Wl,
    W1,
    b1,
    W2,
    b2,
    W3,
    b3,
):
    """Host-side prep: returns (nc, in_maps, b3) ready for run_bass_kernel_spmd."""
    import ml_dtypes

    seq = np.asarray(sequence_embeddings, np.float32)
    lab = np.asarray(label_embeddings, np.float32)
    Wp = np.asarray(Wp, np.float32)
    Wl = np.asarray(Wl, np.float32)
    W1 = np.asarray(W1, np.float32)
    b1 = np.asarray(b1, np.float32)
    W2 = np.asarray(W2, np.float32)
    b2 = np.asarray(b2, np.float32)
    W3 = np.asarray(W3, np.float32)
    b3 = np.asarray(b3, np.float32)

    bf = ml_dtypes.bfloat16

    # Host-side algebraic folds (cheap: 2.1 GFLOP + 36 MFLOP).
    Wfl = W1[:, D:] @ Wl                       # [O, L_DIM]
    hpb = seq @ (W1[:, :D] @ Wp).T + b1        # [B, O]

    # Fold W3 into W2 and permute features so w3>=0 come first.
    w3v = W3[0]
    perm = np.concatenate([np.where(w3v >= 0)[0], np.where(w3v < 0)[0]])
    cut = int((w3v >= 0).sum())
    W2p = (W2 * w3v[:, None])[perm]            # [O, O]
    b2p = (b2 * w3v)[perm]                     # [O]

    wflT = np.ascontiguousarray(
        Wfl.T.reshape(KT, 128, 8, 128).transpose(1, 0, 2, 3)
    ).astype(bf)
    w2T = np.ascontiguousarray(W2p.T.reshape(KT, 128, O).transpose(1, 0, 2)).astype(bf)
    hpbT = np.ascontiguousarray(hpb.T.reshape(KT, 128, B).transpose(1, 0, 2)).astype(
        np.float32
    )
    b2bc = np.ascontiguousarray(np.broadcast_to(b2p, (128, O))).astype(np.float32)

    in_maps = []
    for c in range(N_CORES):
        shard = lab[c * NLC : (c + 1) * NLC]   # [NLC, L_DIM]
        labT = np.ascontiguousarray(
            shard.T.reshape(KT, 128, NLC).transpose(1, 0, 2)
        ).astype(bf)
        in_maps.append(
            {"labT": labT, "wflT": wflT, "w2T": w2T, "hpbT": hpbT, "b2bc": b2bc}
        )

    has_b2 = bool(np.any(b2p))
    key = ("nc", cut, has_b2)
    if key not in _CACHE:
        _CACHE[key] = _build_nc(cut, has_b2)
    nc = _CACHE[key]

    # Host-computed ragged tail: flat cols [COLS_DEV, COLS) of each core,
    # i.e. (b=B-1, last labels of the shard). fp32 exact.
    Wfl32, W2_32, W3_32 = Wfl, W2, W3
    tails = []
    nt = COLS - COLS_DEV
    b_last = COLS_DEV // NLC          # batch row of the tail region
    l0 = COLS_DEV - b_last * NLC      # first label offset in shard
    for c in range(N_CORES):
        shard = lab[c * NLC : (c + 1) * NLC]
        hl_t = shard[l0 : l0 + nt] @ Wfl32.T          # [nt, O]
        h_t = np.maximum(hl_t + hpb[b_last], 0.0)
        y_t = np.maximum(h_t @ W2_32.T + b2, 0.0)
        tails.append((y_t @ W3_32.T)[:, 0].astype(np.float32))
    return nc, in_maps, b3, tails


def gather(results, b3, tails):
    """results[c]["logits"]: [128, NRT], flat sample index rt*128+p; tails[c]
    covers the last COLS-COLS_DEV flat columns (host-computed)."""
    parts = []
    for c in range(N_CORES):
        flat = np.empty(COLS, np.float32)
        flat[:COLS_DEV] = results[c]["logits"].T.ravel()
        flat[COLS_DEV:] = tails[c]
        parts.append(flat.reshape(B, NLC))
    out = np.concatenate(parts, axis=1) + b3[0]
    return out.astype(np.float32)


def kernel(**inputs):
    from concourse.bass_utils import run_bass_kernel_spmd

    nc, in_maps, b3, tails = prepare(**inputs)
    res = run_bass_kernel_spmd(nc, in_maps, core_ids=list(range(N_CORES)))
    return gather(res.results, b3, tails)


# revision 28
# speedup vs baseline: 1.0328x; 1.0100x over previous
"""ProTCL-style dense MLP over a [B=16, NL=5000] cross-join, on 8 TRN2 cores.

Math (reference):
    P_e = seq @ Wp.T; L_e = lab @ Wl.T
    h   = relu(P_e @ W1p.T [+broadcast] L_e @ W1l.T + b1)   # [B, NL, O]
    h   = relu(h @ W2.T + b2)                               # [B, NL, O]
    out = (h @ W3.T + b3)[..., 0]                           # [B, NL]

Strategy:
  - Shard the label axis across 8 cores (625 labels each); host gathers.
  - Host folds (W1l @ Wl) into one matrix Wfl so the device does a single
    matmul for hl = lab @ Wfl.T, and folds the whole (tiny, B=16) sequence
    path into hpb = seq @ (W1p @ Wp).T + b1 on the host.
  - Device works feature-major for h: features on partitions, (b, label)
    sample columns on the free axis.
  - The dominant W2 matmul runs with h STATIONARY and W2 moving, so its
    output z lands sample-major [128 samples, 1024 features]. The W3
    dot-product then becomes a FREE-dim reduction, which the relu pass
    computes for free via the scalar engine's accum_out:
        W3 is folded into W2 on the host (W2'' = diag(W3) W2, rows permuted
        so w3>=0 features come first). Then
        w3_o*relu(y_o) = relu(z_o) for w3_o>=0, and -relu(-z_o) otherwise,
        so logits = sum(relu(z_pos)) - sum(relu(-z_neg)) — ACT passes with
        accum_out, no PE matvec and no bf16 rounding of y.
  - bf16 operands with fp32 PSUM accumulation.
"""

import numpy as np

B = 16
NL = 5000
D = 1024
O = 1024
N_CORES = 8
NLC = NL // N_CORES        # 625 labels per core
KT = O // 128              # 8 contraction tiles
SC = 512                   # sample columns per h-prep super-chunk
COLS = B * NLC             # 10000 sample columns per core
COLS_DEV = (COLS // 128) * 128  # 9984 columns on device (full 128-wide r-tiles)
NRT = COLS_DEV // 128      # 78 sample r-tiles
# the ragged 16-column tail (b=15, last labels of each shard) is computed on
# the host in fp32 — 0.5 GFLOP total, avoids a 97%-idle PE tile per core

_CACHE = {}


def _patch_tile_drain():
    """This container's walrus codegen rejects >1 sync wait per instruction.
    Tile freely attaches one wait per producer domain. Two patches:
      1. _commit_instruction: before committing an instruction carrying N>1
         waits, emit N-1 single-wait NOPs on the same engine (engines execute
         their stream in order, so semantics are identical).
      2. The tail drain (emitted after the commit hook) gets the same
         treatment via _drain_and_barrier."""
    import concourse.mybir as mybir
    import concourse.tile as tile
    from concourse.vector_clock import ScopedClock

    if getattr(tile.TileContext, "_drain_patch_applied", False):
        return

    _orig_commit = tile.TileContext._commit_instruction

    def _commit_instruction(self, inst, lazy_reg_writes: bool = True):
        si = getattr(inst, "sync_info", None)
        if si is not None and len(si.on_wait) > 1:
            waits = list(si.on_wait)
            for w in waits[:-1]:
                nop = mybir.InstNoOp(
                    name=self.nc.get_next_instruction_name(),
                    engine=inst.engine,
                    sync_info=mybir.SyncInfo(on_wait=[w], on_update=[]),
                    bass_nofuse=True,
                )
                self._add_instruction(nop)
            inst.sync_info = mybir.SyncInfo(
                on_wait=[waits[-1]], on_update=list(si.on_update)
            )
        return _orig_commit(self, inst, lazy_reg_writes)

    tile.TileContext._commit_instruction = _commit_instruction

    def _drain_and_barrier(self, tick_clock, wait_clock):
        probe = self.nc.sync.nop(nofuse=True)
        wait_clock.add_sem_waits(
            probe.ins, ScopedClock({None: tick_clock.global_clock})
        )
        si = probe.ins.sync_info
        waits = list(si.on_wait) if si is not None else []
        if len(waits) > 1:
            probe.ins.sync_info = mybir.SyncInfo(
                on_wait=waits[:1], on_update=list(si.on_update)
            )
            for w in waits[1:]:
                extra = self.nc.sync.nop(nofuse=True)
                extra.ins.sync_info = mybir.SyncInfo(on_wait=[w], on_update=[])
        self.nc.sync.drain()
        self.nc.all_engine_barrier()
        assert self.sems is not None
        popped = self.nc._tile_sem_poison_stack.pop()
        assert popped is self._sem_poison
        self.nc.clear_and_free_semaphores(list(self.sems.allocated().values()))
        self.nc.all_engine_barrier()

    tile.TileContext._drain_and_barrier = _drain_and_barrier
    tile.TileContext._drain_patch_applied = True


def _build_nc(cut, has_b2):
    """cut: number of w3>=0 features after the host permutation (static
    split point between the relu(+) and min(,0) accumulation regions).
    has_b2: emit the z += b2 pass (skipped when b2 is identically zero)."""
    import concourse.bass as bass
    import concourse.mybir as mybir
    import concourse.tile as tile

    _patch_tile_drain()

    f32 = mybir.dt.float32
    bf16 = mybir.dt.bfloat16
    Relu = mybir.ActivationFunctionType.Relu
    add = mybir.AluOpType.add
    amax = mybir.AluOpType.max
    sub = mybir.AluOpType.subtract

    nc = bass.Bass("TRN2")

    labT_d = nc.declare_dram_parameter("labT", [128, KT, NLC], bf16, isOutput=False)
    wfl_d = nc.declare_dram_parameter("wflT", [128, KT, 8, 128], bf16, isOutput=False)
    w2_d = nc.declare_dram_parameter("w2T", [128, KT, O], bf16, isOutput=False)
    hpb_d = nc.declare_dram_parameter("hpbT", [128, KT, B], f32, isOutput=False)
    b2_d = nc.declare_dram_parameter("b2bc", [128, O], f32, isOutput=False)
    out_d = nc.declare_dram_parameter("logits", [128, NRT], f32, isOutput=True)

    # per-quarter (256-wide) reduction regions: (offset, width, sign).
    # 256-wide matmuls avoid the full-bank (N=512) PSUM write penalty
    # (~0.51 ns/col vs ~0.41 measured).
    qchunks = [(0, 341), (341, 341), (682, 342)]
    q_regions = []
    for lo, qw in qchunks:
        hi = lo + qw
        regs = []
        p_hi = min(hi, max(lo, cut))
        if p_hi > lo:
            regs.append((0, p_hi - lo, +1))
        n_lo = max(lo, min(hi, cut))
        if hi > n_lo:
            regs.append((n_lo - lo, hi - n_lo, -1))
        q_regions.append(regs)

    with tile.TileContext(nc) as tc:
        with (
            tc.tile_pool(name="const", bufs=1) as cpool,
            tc.tile_pool(name="h", bufs=3) as hpool,
            tc.tile_pool(name="scr", bufs=4) as spool,
            tc.tile_pool(name="acc", bufs=24) as apool,
            tc.tile_pool(name="ps", bufs=7, space="PSUM") as pspool,
        ):
            # Per-k tiles so dependencies (and DMAs) are fine-grained: the
            # first phase-1 matmul only waits for its own k-slice DMAs.
            labT = [cpool.tile([128, NLC], bf16, tag=f"labT{k}", name=f"labT{k}") for k in range(KT)]
            wfl = [cpool.tile([128, 8, 128], bf16, tag=f"wfl{k}", name=f"wfl{k}") for k in range(KT)]
            for k in range(KT):
                nc.sync.dma_start(labT[k][:], labT_d[:, k])
                nc.sync.dma_start(wfl[k][:], wfl_d[:, k])
            w2 = [cpool.tile([128, O], bf16, tag=f"w2{k}", name=f"w2{k}") for k in range(KT)]
            for k in range(KT):
                nc.sync.dma_start(w2[k][:], w2_d[:, k])
            hpb = cpool.tile([128, KT, B], f32)
            nc.sync.dma_start(hpb[:], hpb_d[:])
            if has_b2:
                b2bc = cpool.tile([128, O], f32)
                nc.sync.dma_start(b2bc[:], b2_d[:])

            logits_part = cpool.tile([128, NRT], f32)
            hlT = [cpool.tile([128, NLC], bf16, tag=f"hlT{k}", name=f"hlT{k}") for k in range(KT)]

            # ---- phase 0: PE warmup while the remaining DMAs stream ----
            # Uses labT[0] (the first DMA to land, ~2.5us) as junk operands so
            # nothing has to be zero-filled first; output PSUM is never read.
            # ~16 matmuls = ~3.4us keeps the HAM window busy so phase 1 starts
            # at full clock as soon as wfl[0] arrives.
            wps = pspool.tile([128, 512], f32, tag="ps", name="warm_ps")
            for _ in range(9):
                nc.tensor.matmul(
                    wps[:, :500],
                    lhsT=labT[0][:, :128],
                    rhs=labT[0][:, :500],
                    start=True,
                    stop=True,
                )

            # ---- phase 1: hlT[m][l] = (Wfl @ labT)[m-tile] ----
            # k-outer over half the m-tiles at a time (4 PSUM banks), so the
            # first matmul starts as soon as the k=0 slices have landed.
            for n0, w in ((0, 500), (500, NLC - 500)):
                for mhalf in (0, 4):
                    pss = [
                        pspool.tile([128, 512], f32, tag="ps", name=f"ps1_{n0}_{mhalf}_{i}")
                        for i in range(4)
                    ]
                    for k in range(KT):
                        for mi in range(4):
                            m = mhalf + mi
                            nc.tensor.matmul(
                                pss[mi][:, :w],
                                lhsT=wfl[k][:, m, :],
                                rhs=labT[k][:, n0 : n0 + w],
                                start=(k == 0),
                                stop=(k == KT - 1),
                            )
                    for mi in range(4):
                        m = mhalf + mi
                        if m % 2 == 0:
                            nc.vector.tensor_copy(hlT[m][:, n0 : n0 + w], pss[mi][:, :w])
                        else:
                            nc.scalar.copy(hlT[m][:, n0 : n0 + w], pss[mi][:, :w])

            # ---- phase 2: h stationary, W2'' moving; relu+reduce on ACT ----
            for sc0 in range(0, COLS_DEV, SC):
                w = min(SC, COLS_DEV - sc0)
                segs = []
                for b in range(sc0 // NLC, (sc0 + w - 1) // NLC + 1):
                    lo = max(sc0, b * NLC)
                    hi = min(sc0 + w, (b + 1) * NLC)
                    segs.append((b, lo, hi))

                h_sb = hpool.tile([128, KT, SC], bf16, tag="h", name=f"h_{sc0}")
                for k in range(KT):
                    for b, lo, hi in segs:
                        nc.vector.tensor_scalar(
                            h_sb[:, k, lo - sc0 : hi - sc0],
                            hlT[k][:, lo - b * NLC : hi - b * NLC],
                            hpb[:, k, b : b + 1],
                            0.0,
                            add,
                            amax,
                        )

                for r0 in range(0, w, 128):
                    m = min(128, w - r0)
                    rt = (sc0 + r0) // 128
                    accs = []
                    for q, (qlo, qw) in enumerate(qchunks):
                        ps = pspool.tile(
                            [128, 342], f32, tag="ps", name=f"ps_{rt}_{q}"
                        )
                        for k in range(KT):
                            nc.tensor.matmul(
                                ps[:m, :qw],
                                lhsT=h_sb[:, k, r0 : r0 + m],
                                rhs=w2[k][:, qlo : qlo + qw],
                                start=(k == 0),
                                stop=(k == KT - 1),
                            )
                        if has_b2:
                            # z += b2'' (broadcast rows materialized host-side)
                            nc.vector.tensor_tensor(
                                ps[:m, :qw],
                                ps[:m, :qw],
                                b2bc[:m, qlo : qlo + qw],
                                add,
                            )
                        for off, wr, sign in q_regions[q]:
                            acc = apool.tile(
                                [128, 1], f32, tag="acc", name=f"acc_{rt}_{q}_{off}"
                            )
                            if sign > 0:
                                # sum(w3*relu(y)) over w3>=0 rows = sum(relu(z))
                                scr = spool.tile(
                                    [128, 342], bf16, tag="scr", name=f"scr_{rt}_{q}_{off}"
                                )
                                nc.scalar.activation(
                                    scr[:m, :wr],
                                    ps[:m, off : off + wr],
                                    Relu,
                                    accum_out=acc[:m],
                                )
                            else:
                                # sum over w3<0 rows = sum(min(z, 0))
                                scr = spool.tile(
                                    [128, 342], bf16, tag="vscr", name=f"vscr_{rt}_{q}_{off}"
                                )
                                nc.vector.tensor_scalar(
                                    scr[:m, :wr],
                                    ps[:m, off : off + wr],
                                    0.0,
                                    0.0,
                                    mybir.AluOpType.min,
                                    add,
                                    accum_out=acc[:m],
                                )
                            accs.append(acc)

                    # logits[rt] = sum of all accumulators
                    t = apool.tile([128, 1], f32, tag="acc", name=f"sum_{rt}")
                    nc.vector.tensor_tensor(t[:m], accs[0][:m], accs[1][:m], add)
                    for a in accs[2:-1]:
                        nc.vector.tensor_tensor(t[:m], t[:m], a[:m], add)
                    nc.vector.tensor_tensor(
                        logits_part[:m, rt : rt + 1], t[:m], accs[-1][:m], add
                    )
                    if rt == 47:
                        nc.sync.dma_start(out_d[:, :48], logits_part[:, :48])
                    elif rt == NRT - 1:
                        nc.sync.dma_start(out_d[:, 48:], logits_part[:, 48:])


    return nc


def prepare(
    sequence_embeddings,
    label_embeddings,
    Wp,
    Wl,
    W1,
    b1,
    W2,
    b2,
    W3,
    b3,
):
    """Host-side prep: returns (nc, in_maps, b3) ready for run_bass_kernel_spmd."""
    import ml_dtypes

    seq = np.asarray(sequence_embeddings, np.float32)
    lab = np.asarray(label_embeddings, np.float32)
    Wp = np.asarray(Wp, np.float32)
    Wl = np.asarray(Wl, np.float32)
    W1 = np.asarray(W1, np.float32)
    b1 = np.asarray(b1, np.float32)
    W2 = np.asarray(W2, np.float32)
    b2 = np.asarray(b2, np.float32)
    W3 = np.asarray(W3, np.float32)
    b3 = np.asarray(b3, np.float32)

    bf = ml_dtypes.bfloat16

    # Host-side algebraic folds (cheap: 2.1 GFLOP + 36 MFLOP).
    Wfl = W1[:, D:] @ Wl                       # [O, L_DIM]
    hpb = seq @ (W1[:, :D] @ Wp).T + b1        # [B, O]

    # Fold W3 into W2 and permute features so w3>=0 come first.
    w3v = W3[0]
    perm = np.concatenate([np.where(w3v >= 0)[0], np.where(w3v < 0)[0]])
    cut = int((w3v >= 0).sum())
    W2p = (W2 * w3v[:, None])[perm]            # [O, O]
    b2p = (b2 * w3v)[perm]                     # [O]

    wflT = np.ascontiguousarray(
        Wfl.T.reshape(KT, 128, 8, 128).transpose(1, 0, 2, 3)
    ).astype(bf)
    w2T = np.ascontiguousarray(W2p.T.reshape(KT, 128, O).transpose(1, 0, 2)).astype(bf)
    hpbT = np.ascontiguousarray(hpb.T.reshape(KT, 128, B).transpose(1, 0, 2)).astype(
        np.float32
    )
    b2bc = np.ascontiguousarray(np.broadcast_to(b2p, (128, O))).astype(np.float32)

    in_maps = []
    for c in range(N_CORES):
        shard = lab[c * NLC : (c + 1) * NLC]   # [NLC, L_DIM]
        labT = np.ascontiguousarray(
            shard.T.reshape(KT, 128, NLC).transpose(1, 0, 2)
        ).astype(bf)
        in_maps.append(
            {"labT": labT, "wflT": wflT, "w2T": w2T, "hpbT": hpbT, "b2bc": b2bc}
        )

    has_b2 = bool(np.any(b2p))
    key = ("nc", cut, has_b2)
    if key not in _CACHE:
        _CACHE[key] = _build_nc(cut, has_b2)
    nc = _CACHE[key]

    # Host-computed ragged tail: flat cols [COLS_DEV, COLS) of each core,
    # i.e. (b=B-1, last labels of the shard). fp32 exact.
    Wfl32, W2_32, W3_32 = Wfl, W2, W3
    tails = []
    nt = COLS - COLS_DEV
    b_last = COLS_DEV // NLC          # batch row of the tail region
    l0 = COLS_DEV - b_last * NLC      # first label offset in shard
    for c in range(N_CORES):
        shard = lab[c * NLC : (c + 1) * NLC]
        hl_t = shard[l0 : l0 + nt] @ Wfl32.T          # [nt, O]
        h_t = np.maximum(hl_t + hpb[b_last], 0.0)
        y_t = np.maximum(h_t @ W2_32.T + b2, 0.0)
        tails.append((y_t @ W3_32.T)[:, 0].astype(np.float32))
    return nc, in_maps, b3, tails


def gather(results, b3, tails):
    """results[c]["logits"]: [128, NRT], flat sample index rt*128+p; tails[c]
    covers the last COLS-COLS_DEV flat columns (host-computed)."""
    parts = []
    for c in range(N_CORES):
        flat = np.empty(COLS, np.float32)
        flat[:COLS_DEV] = results[c]["logits"].T.ravel()
        flat[COLS_DEV:] = tails[c]
        parts.append(flat.reshape(B, NLC))
    out = np.concatenate(parts, axis=1) + b3[0]
    return out.astype(np.float32)


def kernel(**inputs):
    from concourse.bass_utils import run_bass_kernel_spmd

    nc, in_maps, b3, tails = prepare(**inputs)
    res = run_bass_kernel_spmd(nc, in_maps, core_ids=list(range(N_CORES)))
    return gather(res.results, b3, tails)
